# revision 50
# baseline (speedup 1.0000x reference)
"""3-layer GAT on 8 Trainium2 NeuronCores (Bass/Tile) — v4.

Pool-engine-lean design. The SWDGE dma_gather generation is the scarce
resource (~3.4us fixed/call, <=~1000 indices/call HW limit):
  - nodes block-mapped into NG=2 row blocks (int16 gather halves); ONE idx
    table serves the x-gather (layer 1) and the hs-table gathers (2/3).
  - one gather per (tile, block): ~98 calls/layer, nothing else on Pool.
  - layer-1 h/s come out of the per-edge transform of gathered raw x rows
    (512B/row instead of 2304B feature rows; no feature AllGather at all).
  - per-edge dst logits via PE transpose of the eq selection matrix + a
    tiny matmul against persistent SBUF d tables (written directly by the
    epilogues): no d gathers, no DRAM d tables.
  - layer-2/3 feature tables all-gathered per block (2 collectives each,
    RDH ~100-200GB/s), firing as the producing half completes.
  - emission is software-pipelined so the PE interleaves eq transposes
    with the previous tile's aggregation matmuls.

Self-contained: only imports the system concourse install.
"""

import os
import sys

for _p in ("/opt/trn_rl_repo", "/root/.axon_site/_ro/trn_rl_repo"):
    if os.path.isdir(_p) and _p not in sys.path:
        sys.path.insert(0, _p)

from dataclasses import dataclass

import ml_dtypes
import numpy as np

import concourse.bacc as bacc
import concourse.bass as bass
import concourse.tile as tile
from concourse import mybir
from concourse.bass_utils import run_bass_kernel_spmd

P = 128
BF16 = mybir.dt.bfloat16
F32 = mybir.dt.float32
I16 = mybir.dt.int16
AL = mybir.AluOpType
AF = mybir.ActivationFunctionType

NEG_SLOPE_ATT = 0.2
NEG_SLOPE_ACT = 0.01
LN_EPS = 1e-5
MAX_GIDX = 896           # dma_gather HW limit: <=~1000 indices per call


def _ceil(a, b):
    return -(-a // b)


def _pad_elem(n_f32_elems):
    """bf16 row length (elements) padded so row bytes are a multiple of 256."""
    return _ceil(n_f32_elems * 2, 256) * 128


@dataclass
class Cfg:
    N: int = 50000
    E: int = 400000
    F_IN: int = 256
    HEADS: int = 4
    C1: int = 256
    C2: int = 128
    NCLS: int = 32
    NCORES: int = 8

    def __post_init__(self):
        assert self.N % self.NCORES == 0
        self.NL = self.N // self.NCORES
        self.T = _ceil(self.NL, P)
        self.NLP = self.T * P
        self.NPTOT = self.NLP * self.NCORES
        self.NG = 2
        # asymmetric: block 0's (large) AllGather fires mid-L1 and hides
        # behind the remaining tiles; only block 1's smaller AG sits at
        # the L1->L2 boundary.
        self.BLK_T = [29, 20]
        assert sum(self.BLK_T) == self.T
        self.SBT = [0, 29]
        self.GROWS = [b * P * self.NCORES for b in self.BLK_T]
        self.GBASE = [0, self.GROWS[0]]
        assert max(self.GROWS) < 32768  # int16 gather indices per group
        H = self.HEADS
        self.CO1 = H * self.C1
        self.CO2 = H * self.C2
        assert self.F_IN % P == 0 and self.CO1 % P == 0 and self.CO2 % P == 0
        self.ELEM2 = _pad_elem(self.CO2 + H)          # [h2|s2|pad] rows
        self.ELEM3 = _pad_elem(self.NCLS + 1)         # [h3|s3|pad] rows
        self.W1w = self.CO1 + 2 * H                   # [W1 | U_s | U_d]
        self.W2w = self.CO2 + 2 * H
        self.W3w = self.NCLS + 2


@dataclass
class Meta:
    nch: list   # [T][NG] chunk counts (common across cores)
    sig: list   # [NG][T] idx col offsets (group-major)
    sc: list    # [T][NG] dstloc column offsets (tile-major)
    SI: int
    SC: int


def _grp_map(cfg: Cfg, core, loc):
    """(source block, within-block row index) for node (core, local idx)."""
    t = loc // P
    b = (t >= cfg.BLK_T[0]).astype(np.int64)
    blk_t = np.array(cfg.BLK_T)[b]
    sb = np.array(cfg.SBT)[b]
    return b, core * blk_t * P + (loc - sb * P)


def _gidx_map(cfg: Cfg, core, loc):
    b, off = _grp_map(cfg, core, loc)
    return np.array(cfg.GBASE)[b] + off


def host_prep(cfg: Cfg, x, edge_src, edge_dst,
              W1, a_src1, a_dst1, b1, ln1_g, ln1_b,
              W2, a_src2, a_dst2, b2, ln2_g, ln2_b,
              W3, a_src3, a_dst3, b3, ln3_g, ln3_b):
    c = cfg
    bf = ml_dtypes.bfloat16

    # ---- append self loops, shard edges by destination core
    loops = np.arange(c.N, dtype=np.int64)
    src = np.concatenate([edge_src.astype(np.int64), loops])
    dst = np.concatenate([edge_dst.astype(np.int64), loops])

    dst_core = dst // c.NL
    dstloc = dst - dst_core * c.NL
    tile_id = dstloc // P
    grp, idx16 = _grp_map(c, src // c.NL, src % c.NL)
    grp = grp.astype(np.int64)
    idx16 = idx16.astype(np.int64)
    NG = c.NG

    counts = np.zeros((c.NCORES, c.T, NG), np.int64)
    np.add.at(counts, (dst_core, tile_id, grp), 1)
    nch = np.maximum(_ceil(counts.max(axis=0), P), 0)  # [T,NG] chunks
    assert nch.max() * P <= MAX_GIDX, nch.max()
    sig = np.zeros((NG, c.T), np.int64)
    acc = 0
    for g in range(NG):
        for t in range(c.T):
            sig[g, t] = acc
            acc += int(nch[t, g]) * (P // 16)
    SI = int(acc)
    sc = np.zeros((c.T, NG), np.int64)
    acc_sc = 0
    for t in range(c.T):
        for g in range(NG):
            sc[t, g] = acc_sc
            acc_sc += int(nch[t, g])
    SC = int(acc_sc)
    meta = Meta(nch=nch.tolist(), sig=sig.tolist(), sc=sc.tolist(),
                SI=SI, SC=SC)

    order = np.lexsort((grp, tile_id, dst_core))
    src_s = idx16[order]
    dstrel_s = (dstloc - tile_id * P)[order]

    starts = np.zeros((c.NCORES, c.T, NG), np.int64)
    run = 0
    for cc in range(c.NCORES):
        for t in range(c.T):
            for g in range(NG):
                starts[cc, t, g] = run
                run += int(counts[cc, t, g])

    idx_tabs, dl_tabs = [], []
    for cc in range(c.NCORES):
        itab = np.zeros((16, SI), np.int16)
        dtab = np.full((P, SC), -1.0, np.float32)
        for t in range(c.T):
            for g in range(NG):
                m = int(counts[cc, t, g])
                n = int(nch[t, g])
                if n == 0:
                    continue
                s0 = int(starts[cc, t, g])
                iv = np.zeros(n * P, np.int16)
                iv[:m] = src_s[s0:s0 + m].astype(np.int16)
                cols = int(sig[g, t])
                itab[:, cols:cols + n * (P // 16)] = iv.reshape(
                    n * P // 16, 16).T
                dv = np.full(n * P, -1.0, np.float32)
                dv[:m] = dstrel_s[s0:s0 + m].astype(np.float32)
                dtab[:, sc[t, g]:sc[t, g] + n] = dv.reshape(n, P).T
        idx_tabs.append(np.tile(itab, (8, 1)))
        dl_tabs.append(dtab)

    # ---- block-mapped full x table (replicated to every core)
    xfull = np.zeros((c.NPTOT, c.F_IN), np.float32)
    for cc in range(c.NCORES):
        loc = np.arange(c.NL)
        gi = _gidx_map(c, np.full(c.NL, cc), loc)
        xfull[gi] = x[cc * c.NL:(cc + 1) * c.NL]
    xfull = xfull.astype(bf)

    # ---- weights (augmented with U = W.T @ a columns), bf16
    def aug(W, a_s, a_d, H, C):
        WT = W.T.astype(np.float64)
        U_s = np.zeros((WT.shape[0], H))
        U_d = np.zeros((WT.shape[0], H))
        for h in range(H):
            U_s[:, h] = WT[:, h * C:(h + 1) * C] @ a_s[h].astype(np.float64)
            U_d[:, h] = WT[:, h * C:(h + 1) * C] @ a_d[h].astype(np.float64)
        return np.concatenate([WT, U_s, U_d], axis=1).astype(bf)

    W1a = aug(W1, a_src1, a_dst1, c.HEADS, c.C1)   # [F_IN, CO1+2H]
    W2a = aug(W2, a_src2, a_dst2, c.HEADS, c.C2)   # [CO1, CO2+2H]
    W3a = aug(W3, a_src3, a_dst3, 1, c.NCLS)       # [CO2, NCLS+2]

    def bln(b, g, be):
        row = np.concatenate([b, g, be]).astype(np.float32)[None, :]
        return np.repeat(row, P, axis=0)

    bln1 = bln(b1, ln1_g, ln1_b)
    bln2 = bln(b2, ln2_g, ln2_b)
    bln3 = bln(b3, ln3_g, ln3_b)

    ident = np.eye(P, dtype=bf)
    iota_f = np.repeat(np.arange(P, dtype=np.float32)[None, :], P, axis=0)

    in_maps = []
    for cc in range(c.NCORES):
        xl = np.zeros((c.NLP, c.F_IN), np.float32)
        xl[:c.NL] = x[cc * c.NL:(cc + 1) * c.NL]
        in_maps.append({
            "xT": np.ascontiguousarray(xl.T).astype(bf),
            "xfull": xfull,
            "W1a": W1a, "W2a": W2a, "W3a": W3a,
            "bln1": bln1, "bln2": bln2, "bln3": bln3,
            "idx16": idx_tabs[cc], "dstloc": dl_tabs[cc],
            "iotaf": iota_f, "ident": ident,
        })
    return in_maps, meta


# --------------------------------------------------------------------------
# device program
# --------------------------------------------------------------------------

def build_nc(cfg: Cfg, meta: Meta):
    c = cfg
    H = c.HEADS
    nc = bacc.Bacc("TRN2", target_bir_lowering=False, debug=False,
                   num_devices=c.NCORES, enable_partition_id=False)

    # ---- I/O
    xT = nc.dram_tensor("xT", [c.F_IN, c.NLP], BF16, kind="ExternalInput").ap()
    xfull = nc.dram_tensor("xfull", [c.NPTOT, c.F_IN], BF16,
                           kind="ExternalInput").ap()
    W1a = nc.dram_tensor("W1a", [c.F_IN, c.W1w], BF16, kind="ExternalInput").ap()
    W2a = nc.dram_tensor("W2a", [c.CO1, c.W2w], BF16, kind="ExternalInput").ap()
    W3a = nc.dram_tensor("W3a", [c.CO2, c.W3w], BF16, kind="ExternalInput").ap()
    bln1 = nc.dram_tensor("bln1", [P, 3 * c.CO1], F32, kind="ExternalInput").ap()
    bln2 = nc.dram_tensor("bln2", [P, 3 * c.CO2], F32, kind="ExternalInput").ap()
    bln3 = nc.dram_tensor("bln3", [P, 3 * c.NCLS], F32, kind="ExternalInput").ap()
    idx16 = nc.dram_tensor("idx16", [P, meta.SI], I16, kind="ExternalInput").ap()
    dstloc = nc.dram_tensor("dstloc", [P, meta.SC], F32, kind="ExternalInput").ap()
    iotaf = nc.dram_tensor("iotaf", [P, P], F32, kind="ExternalInput").ap()
    ident = nc.dram_tensor("ident", [P, P], BF16, kind="ExternalInput").ap()
    y = nc.dram_tensor("y", [c.NLP, c.NCLS], F32, kind="ExternalOutput").ap()

    groups = [list(range(c.NCORES))]

    def block_ag(loc_t, full_ts):
        for b in range(c.NG):
            r0 = c.SBT[b] * P
            nr = c.BLK_T[b] * P
            nc.gpsimd.collective_compute(
                "AllGather", AL.bypass, replica_groups=groups,
                ins=[loc_t[r0:r0 + nr, :].opt()],
                outs=[full_ts[b][:].opt()])

    with tile.TileContext(nc) as tc:
        dram_cm = tc.tile_pool(name="dram", bufs=1, space="DRAM")
        dram = dram_cm.__enter__()
        hs2_loc = dram.tile([c.NLP, c.ELEM2], BF16)
        hs2_full = [dram.tile([c.GROWS[b], c.ELEM2], BF16, addr_space="Shared",
                              name=f"hs2f{b}") for b in range(c.NG)]
        hs3_loc = dram.tile([c.NLP, c.ELEM3], BF16)
        hs3_full = [dram.tile([c.GROWS[b], c.ELEM3], BF16, addr_space="Shared",
                              name=f"hs3f{b}") for b in range(c.NG)]

        # ---- persistent SBUF constants
        cpool_cm = tc.tile_pool(name="const", bufs=1)
        cpool = cpool_cm.__enter__()
        KC1 = c.F_IN // P
        W1a_sb = cpool.tile([P, KC1 * c.W1w], BF16)
        for k in range(KC1):
            nc.sync.dma_start(W1a_sb[:, k * c.W1w:(k + 1) * c.W1w],
                              W1a[k * P:(k + 1) * P, :])
        KC2 = c.CO1 // P
        W2a_sb = cpool.tile([P, KC2 * c.W2w], BF16)
        for k in range(KC2):
            nc.sync.dma_start(W2a_sb[:, k * c.W2w:(k + 1) * c.W2w],
                              W2a[k * P:(k + 1) * P, :])
        KC3 = c.CO2 // P
        W3a_sb = cpool.tile([P, KC3 * c.W3w], BF16)
        for k in range(KC3):
            nc.sync.dma_start(W3a_sb[:, k * c.W3w:(k + 1) * c.W3w],
                              W3a[k * P:(k + 1) * P, :])
        bln1_sb = cpool.tile([P, 3 * c.CO1], F32)
        nc.sync.dma_start(bln1_sb[:], bln1[:])
        bln2_sb = cpool.tile([P, 3 * c.CO2], F32)
        nc.sync.dma_start(bln2_sb[:], bln2[:])
        bln3_sb = cpool.tile([P, 3 * c.NCLS], F32)
        nc.sync.dma_start(bln3_sb[:], bln3[:])
        idx_sb = cpool.tile([P, meta.SI], I16)
        nc.sync.dma_start(idx_sb[:], idx16[:])
        dl_sb = cpool.tile([P, meta.SC], F32)
        nc.sync.dma_start(dl_sb[:], dstloc[:])
        iota_sb = cpool.tile([P, P], F32)
        nc.sync.dma_start(iota_sb[:], iotaf[:])
        id_sb = cpool.tile([P, P], BF16)
        nc.sync.dma_start(id_sb[:], ident[:])
        # persistent per-layer local d tables (dst logits), bf16
        d1_sb = cpool.tile([P, c.T * H], BF16)
        d2_sb = cpool.tile([P, c.T * H], BF16)
        d3_sb = cpool.tile([P, c.T * 1], BF16)

        # ================= prologue: local d1 logits -> d1_sb
        with (
            tc.tile_pool(name="pro", bufs=3) as pro,
            tc.tile_pool(name="prop", bufs=2, space="PSUM") as prop,
        ):
            for t in range(c.T):
                xt = pro.tile([P, KC1 * P], BF16, tag="xt")
                for k in range(KC1):
                    nc.sync.dma_start(xt[:, k * P:(k + 1) * P],
                                      xT[k * P:(k + 1) * P, t * P:(t + 1) * P])
                dsp = prop.tile([P, H], F32, tag="dsp")
                for k in range(KC1):
                    nc.tensor.matmul(
                        out=dsp[:],
                        lhsT=xt[:, k * P:(k + 1) * P],
                        rhs=W1a_sb[:, k * c.W1w + c.CO1 + H:
                                   k * c.W1w + c.CO1 + 2 * H],
                        start=(k == 0), stop=(k == KC1 - 1))
                nc.vector.tensor_copy(d1_sb[:, t * H:(t + 1) * H], dsp[:])

        def ag2_fire(b):
            r0 = c.SBT[b] * P
            nr = c.BLK_T[b] * P
            nc.gpsimd.collective_compute(
                "AllGather", AL.bypass, replica_groups=groups,
                ins=[hs2_loc[r0:r0 + nr, :].opt()],
                outs=[hs2_full[b][:].opt()])

        # ================= layer 1
        _l1_phase(nc, tc, c, meta, xfull, W1a_sb, W2a_sb, bln1_sb,
                  iota_sb, id_sb, idx_sb, dl_sb, d1_sb, d2_sb, hs2_loc,
                  ag2_fire)

        # ================= layer 2 (+fused L3 transform)
        _edge_phase(
            nc, tc, c, meta, lay=2, Hn=H, Ch=c.C2, ELEM=c.ELEM2,
            hs_full=hs2_full, d_sb=d2_sb, bln_sb=bln2_sb,
            iota_sb=iota_sb, id_sb=id_sb, idx_sb=idx_sb, dl_sb=dl_sb,
            fuse=dict(W_sb=W3a_sb, KC=KC3, Ww=c.W3w, CO=c.NCLS, Hn2=1,
                      ELEMn=c.ELEM3, hs_loc=hs3_loc, d_next=d3_sb),
            final=None, y=None,
            ag_fire=lambda b: nc.gpsimd.collective_compute(
                "AllGather", AL.bypass, replica_groups=groups,
                ins=[hs3_loc[c.SBT[b] * P:
                             (c.SBT[b] + c.BLK_T[b]) * P, :].opt()],
                outs=[hs3_full[b][:].opt()]))

        # ================= layer 3 + log_softmax
        _edge_phase(
            nc, tc, c, meta, lay=3, Hn=1, Ch=c.NCLS, ELEM=c.ELEM3,
            hs_full=hs3_full, d_sb=d3_sb, bln_sb=bln3_sb,
            iota_sb=iota_sb, id_sb=id_sb, idx_sb=idx_sb, dl_sb=dl_sb,
            fuse=None, final=True, y=y, ag_fire=None)

        cpool_cm.__exit__(None, None, None)
        dram_cm.__exit__(None, None, None)

    nc.compile()
    return nc


# --------------------------------------------------------------------------
# layer-1 phase: per-edge transform fused with aggregation
# --------------------------------------------------------------------------

def _l1_phase(nc, tc, c: Cfg, meta: Meta, xfull, W1a_sb, W2a_sb, bln_sb,
              iota_sb, id_sb, idx_sb, dl_sb, d1_sb, d2_sb, hs2_loc,
              ag_fire):
    H = c.HEADS
    CO = c.CO1
    Ch = c.C1
    nch = meta.nch
    max_ntot = max(sum(r) for r in nch)
    max_nch = max(max(r) for r in nch)
    KC1 = c.F_IN // P
    GH_ACT = 640          # PSUM->SBUF copy split: ACT cols, DVE the rest
    DOFF = H + max_ntot * H   # d section start in the sden bank

    with (
        tc.tile_pool(name="gx", bufs=9) as gxp,
        tc.tile_pool(name="gh", bufs=max_ntot + 4) as ghp,
        tc.tile_pool(name="w1", bufs=2) as wp,
        tc.tile_pool(name="s1", bufs=3) as sp,
        tc.tile_pool(name="eqt", bufs=2) as eqtp,
        tc.tile_pool(name="ep", bufs=1) as ep,
        tc.tile_pool(name="x2p", bufs=3) as x2p,
        tc.tile_pool(name="hst1", bufs=2) as hstp,
        tc.tile_pool(name="php", bufs=3, space="PSUM") as php,
        tc.tile_pool(name="pagg", bufs=1, space="PSUM") as pagg,
        tc.tile_pool(name="psd", bufs=2, space="PSUM") as psd,
        tc.tile_pool(name="pscr", bufs=1, space="PSUM") as pscr,
    ):
        state = {}

        def stage_gather(t):
            st = state.setdefault(t, {})
            for g in range(c.NG):
                nb = nch[t][g]
                if nb == 0:
                    continue
                si = meta.sig[g][t]
                nidx = nb * P
                Gx = gxp.tile([P, KC1 * max_nch * P], BF16, tag=f"Gx{g}")
                nc.gpsimd.dma_gather(
                    out_ap=Gx[:, 0:KC1 * nidx].rearrange(
                        "p (j e) -> p j e", e=nidx),
                    in_ap=xfull[c.GBASE[g]:c.GBASE[g] + c.GROWS[g], :],
                    idxs_ap=idx_sb[:, si:si + nb * (P // 16)],
                    num_idxs=nidx, num_idxs_reg=nidx, elem_size=c.F_IN,
                    transpose=True)
                st.setdefault("Gx", {})[g] = Gx

        def stage_transform(t):
            """eq build; per chunk: transform h + s, interleaved with the
            eq transpose + d matmul of the previous chunk."""
            st = state[t]
            ntot = sum(nch[t])
            sden = psd.tile([P, 512], F32, tag="sden")
            st["sden"] = sden
            nc.vector.memset(sden[:, 0:DOFF + max_ntot * H], 0)
            eqa = wp.tile([P, max_ntot * P], BF16, tag="eqa")
            st["eqa"] = eqa
            if ntot == 0:
                return
            c0 = meta.sc[t][0]
            dlv = dl_sb[:, c0:c0 + ntot].to_broadcast([P, ntot, P])
            io = iota_sb[:]
            iob = bass.AP(io.tensor, io.offset,
                          [list(io.ap[0]), [0, ntot], list(io.ap[1])])
            nc.vector.tensor_tensor(
                out=eqa[:, 0:ntot * P].rearrange("p (k d) -> p k d", d=P),
                in0=dlv, in1=iob, op=AL.is_equal)

            eqTs = {}

            def trans(k):
                scr = pscr.tile([P, P], BF16, tag="scr")
                nc.tensor.transpose(out=scr[:],
                                    in_=eqa[:, k * P:(k + 1) * P],
                                    identity=id_sb[:])
                eqT = eqtp.tile([P, P], BF16, tag="eqT")
                # alternate the PSUM->SBUF copy between DVE and ACT so
                # neither engine eats the whole ~1470-copy bill
                if k % 2 == 0:
                    nc.vector.tensor_copy(eqT[:], scr[:])
                else:
                    nc.scalar.copy(eqT[:], scr[:])
                eqTs[k] = eqT

            def dmm(k):
                nc.tensor.matmul(
                    out=sden[:, DOFF + k * H:DOFF + (k + 1) * H],
                    lhsT=eqTs.pop(k)[:], rhs=d1_sb[:, t * H:(t + 1) * H],
                    start=False, stop=True, skip_group_check=True)

            Ghs = []
            ci = 0
            for g in range(c.NG):
                if nch[t][g] == 0:
                    continue
                Gx = st["Gx"][g]
                nb = nch[t][g]
                Gxv = Gx[:, 0:KC1 * nb * P].rearrange(
                    "p (j e) -> p j e", e=nb * P)
                for ck in range(nb):
                    Gh = ghp.tile([P, CO], BF16, tag="gh")
                    for half in range(2):
                        hp = php.tile([P, 512], F32, tag="hp")
                        n0 = half * 512
                        for j in range(KC1):
                            nc.tensor.matmul(
                                out=hp[:],
                                lhsT=Gxv[:, j, ck * P:(ck + 1) * P],
                                rhs=W1a_sb[:, j * c.W1w + n0:
                                           j * c.W1w + n0 + 512],
                                start=(j == 0), stop=(j == KC1 - 1))
                        if half == 0:
                            nc.scalar.copy(Gh[:, 0:512], hp[:])
                        else:
                            nc.scalar.copy(Gh[:, 512:GH_ACT],
                                           hp[:, 0:GH_ACT - 512])
                            nc.vector.tensor_copy(Gh[:, GH_ACT:CO],
                                                  hp[:, GH_ACT - 512:512])
                    for j in range(KC1):
                        nc.tensor.matmul(
                            out=sden[:, H + ci * H:H + (ci + 1) * H],
                            lhsT=Gxv[:, j, ck * P:(ck + 1) * P],
                            rhs=W1a_sb[:, j * c.W1w + CO:j * c.W1w + CO + H],
                            start=False, stop=(j == KC1 - 1),
                            skip_group_check=True)
                    trans(ci)
                    if ci >= 1:
                        dmm(ci - 1)
                    Ghs.append(Gh)
                    ci += 1
            dmm(ntot - 1)
            st["Gh"] = Ghs

        def stage_w_agg(t):
            st = state[t]
            ntot = sum(nch[t])
            agg = pagg.tile([P, CO], F32, tag="agg")
            st["agg"] = agg
            sden = st["sden"]
            st["den_ap"] = sden[:, 0:H]
            if ntot == 0:
                nc.vector.memset(agg[:], 0)
                return
            eqa = st["eqa"]
            ssb = wp.tile([P, max_ntot * H], F32, tag="ssb")
            nc.vector.tensor_copy(ssb[:, 0:ntot * H], sden[:, H:H + ntot * H])
            tsda = wp.tile([P, max_ntot * H], F32, tag="tsda")
            nc.vector.tensor_tensor(
                out=tsda[:, 0:ntot * H],
                in0=ssb[:, 0:ntot * H],
                in1=sden[:, DOFF:DOFF + ntot * H], op=AL.add)
            lra = wp.tile([P, max_ntot * H], F32, tag="lra")
            nc.vector.scalar_tensor_tensor(
                out=lra[:, 0:ntot * H], in0=tsda[:, 0:ntot * H],
                scalar=NEG_SLOPE_ATT, in1=tsda[:, 0:ntot * H],
                op0=AL.mult, op1=AL.max)
            wfa = wp.tile([P, max_ntot * H], F32, tag="wfa")
            nc.scalar.activation(wfa[:, 0:ntot * H], lra[:, 0:ntot * H], AF.Exp)
            wfb = wp.tile([P, max_ntot * H], BF16, tag="wfb")
            nc.vector.tensor_copy(wfb[:, 0:ntot * H], wfa[:, 0:ntot * H])

            for k in range(ntot):
                first, last = (k == 0), (k == ntot - 1)
                eq = eqa[:, k * P:(k + 1) * P]
                wf = wfa[:, k * H:(k + 1) * H]
                S = sp.tile([P, H * P], BF16, tag="S")
                eq_b = bass.AP(eq.tensor, eq.offset,
                               [list(eq.ap[0]), [0, H], list(eq.ap[1])])
                nc.vector.tensor_tensor(
                    out=S[:].rearrange("p (h d) -> p h d", h=H),
                    in0=eq_b, in1=wf.to_broadcast([P, H, P]), op=AL.mult)
                Gh = st["Gh"][k]
                for h in range(H):
                    nc.tensor.matmul(
                        out=agg[:, h * Ch:(h + 1) * Ch],
                        lhsT=S[:, h * P:(h + 1) * P],
                        rhs=Gh[:, h * Ch:(h + 1) * Ch],
                        start=first and (h * Ch) % 512 == 0,
                        stop=last and ((h + 1) * Ch) % 512 == 0)
                nc.tensor.matmul(out=sden[:, 0:H], lhsT=eq,
                                 rhs=wfb[:, k * H:(k + 1) * H],
                                 start=False, stop=last,
                                 skip_group_check=True)

        def stage_ln(t):
            st = state[t]
            agg, den_ap = st["agg"], st["den_ap"]
            denr = ep.tile([P, H], F32, tag="denr")
            nc.vector.tensor_scalar(out=denr[:], in0=den_ap, scalar1=1e-16,
                                    scalar2=None, op0=AL.add)
            rec = ep.tile([P, H], F32, tag="rec")
            nc.vector.reciprocal(rec[:], denr[:])
            ob = ep.tile([P, CO], BF16, tag="ob")
            for h in range(H):
                nc.vector.scalar_tensor_tensor(
                    out=ob[:, h * Ch:(h + 1) * Ch],
                    in0=agg[:, h * Ch:(h + 1) * Ch],
                    scalar=rec[:, h:h + 1], op0=AL.mult,
                    in1=bln_sb[:, h * Ch:(h + 1) * Ch], op1=AL.add)
            rs = ep.tile([P, 1], F32, tag="rs")
            nc.vector.tensor_reduce(out=rs[:], in_=ob[:],
                                    axis=mybir.AxisListType.X, op=AL.add)
            nm = ep.tile([P, 1], F32, tag="nm")
            nc.vector.tensor_scalar(out=nm[:], in0=rs[:], scalar1=-1.0 / CO,
                                    scalar2=None, op0=AL.mult)
            sqd = ep.tile([P, CO], BF16, tag="sqd")
            vs = ep.tile([P, 1], F32, tag="vs")
            nc.scalar.activation(sqd[:], ob[:], AF.Square, bias=nm[:, 0:1],
                                 accum_out=vs[:])
            vstd = ep.tile([P, 1], F32, tag="vstd")
            nc.vector.tensor_scalar(out=vstd[:], in0=vs[:], scalar1=1.0 / CO,
                                    scalar2=LN_EPS, op0=AL.mult, op1=AL.add)
            sd = ep.tile([P, 1], F32, tag="sd")
            nc.scalar.activation(sd[:], vstd[:], AF.Sqrt)
            rstd = ep.tile([P, 1], F32, tag="rstd")
            nc.vector.reciprocal(rstd[:], sd[:])
            xcs = ep.tile([P, CO], BF16, tag="xcs")
            nc.vector.tensor_scalar(out=xcs[:], in0=ob[:],
                                    scalar1=nm[:, 0:1], scalar2=rstd[:, 0:1],
                                    op0=AL.add, op1=AL.mult)
            y1 = ep.tile([P, CO], BF16, tag="y1")
            nc.vector.tensor_tensor(out=y1[:], in0=xcs[:],
                                    in1=bln_sb[:, CO:2 * CO], op=AL.mult)
            y2 = ep.tile([P, CO], BF16, tag="y2")
            nc.vector.tensor_tensor(out=y2[:], in0=y1[:],
                                    in1=bln_sb[:, 2 * CO:3 * CO], op=AL.add)
            x2 = x2p.tile([P, CO], BF16, tag="x2")
            nc.vector.scalar_tensor_tensor(
                out=x2[:], in0=y2[:], scalar=NEG_SLOPE_ACT, in1=y2[:],
                op0=AL.mult, op1=AL.max)
            st["x2"] = x2

        def stage_tail(t):
            st = state[t]
            x2 = st["x2"]
            KCn = CO // P
            xt2 = ep.tile([P, KCn * P], BF16, tag="xt2")
            for k in range(KCn):
                scr = pscr.tile([P, P], BF16, tag="scr")
                nc.tensor.transpose(out=scr[:], in_=x2[:, k * P:(k + 1) * P],
                                    identity=id_sb[:])
                nc.scalar.copy(xt2[:, k * P:(k + 1) * P], scr[:])
            hp = php.tile([P, 512], F32, tag="hp")
            hp2 = php.tile([P, 512], F32, tag="hp")
            _mm_splits(nc, hp[:], xt2, W2a_sb, KCn, c.W2w, P, upto=512)
            _mm_splits(nc, hp2[:, 0:c.W2w - 512], xt2, W2a_sb, KCn, c.W2w, P,
                       frm=512)
            _store_hs(nc, hstp, hp[:, 0:c.CO2], hp2[:, 0:H],
                      hp2[:, H:2 * H], c.CO2, c.HEADS, c.ELEM2,
                      hs2_loc, d2_sb, t)
            del state[t]

        def do_tail(tt):
            stage_tail(tt)
            for b in range(c.NG):
                if tt == c.SBT[b] + c.BLK_T[b] - 1:
                    ag_fire(b)

        GD = 6  # gather prefetch depth: rides out the block-0 AG stall
        for t0 in range(min(GD, c.T)):
            stage_gather(t0)
        for t in range(c.T):
            if t + GD < c.T:
                stage_gather(t + GD)
            stage_transform(t)
            stage_w_agg(t)
            stage_ln(t)
            if t - 2 >= 0:
                do_tail(t - 2)
        do_tail(c.T - 2)
        do_tail(c.T - 1)


def _mm_splits(nc, out_ps, lhs_sb, w_sb, KC, Ww, Plhs, frm=0, upto=None):
    """out_ps = sum_k lhs_k.T @ W_k[:, frm:upto], N split at 512."""
    upto = Ww if upto is None else min(upto, Ww)
    splits = []
    n0 = frm
    while n0 < upto:
        nsz = min(512, upto - n0)
        splits.append((n0, nsz))
        n0 += nsz
    for k in range(KC):
        for (n0, nsz) in splits:
            nc.tensor.matmul(
                out=out_ps[:, n0 - frm:n0 - frm + nsz],
                lhsT=lhs_sb[:, k * Plhs:(k + 1) * Plhs],
                rhs=w_sb[:, k * Ww + n0:k * Ww + n0 + nsz],
                start=(k == 0), stop=(k == KC - 1))


def _store_hs(nc, pool, h_ap, s_ap, d_ap, CO, Hn, ELEM, hs_loc, d_next_sb, t):
    """PSUM views -> bf16 [h|s|pad] row tile + SBUF d-table slice."""
    hst = pool.tile([P, ELEM], BF16, tag="hst")
    nc.scalar.copy(hst[:, 0:CO], h_ap)
    nc.vector.tensor_copy(hst[:, CO:CO + Hn], s_ap)
    if ELEM > CO + Hn:
        nc.vector.memset(hst[:, CO + Hn:ELEM], 0)
    nc.vector.tensor_copy(d_next_sb[:, t * Hn:(t + 1) * Hn], d_ap)
    nc.sync.dma_start(hs_loc[t * P:(t + 1) * P, :], hst[:])


# --------------------------------------------------------------------------
# layers 2/3: table-gather edge phase (tile-pipelined, eqT d logits)
# --------------------------------------------------------------------------

def _edge_phase(nc, tc, c: Cfg, meta: Meta, lay, Hn, Ch, ELEM, hs_full, d_sb,
                bln_sb, iota_sb, id_sb, idx_sb, dl_sb, fuse, final, y,
                ag_fire=None):
    CO = Hn * Ch
    nch = meta.nch
    max_ntot = max(sum(r) for r in nch)
    max_nch = max(max(r) for r in nch)
    merge_den = (Hn == 1)

    with (
        tc.tile_pool(name=f"sb{lay}", bufs=2) as sb,
        tc.tile_pool(name=f"sc{lay}", bufs=4) as sbc,
        tc.tile_pool(name=f"sS{lay}", bufs=max_ntot + 2) as sSp,
        tc.tile_pool(name=f"g{lay}", bufs=4) as gp,
        tc.tile_pool(name=f"eqt{lay}", bufs=3) as eqtp,
        tc.tile_pool(name=f"ps{lay}", bufs=1, space="PSUM") as ps1,
        tc.tile_pool(name=f"psagg{lay}", bufs=2, space="PSUM") as psA,
        tc.tile_pool(name=f"psd{lay}", bufs=2, space="PSUM") as psd,
        tc.tile_pool(name=f"pscr{lay}", bufs=1, space="PSUM") as pscr,
    ):
        state = {}

        def stage_gather(t):
            st = state.setdefault(t, {})
            for g in range(c.NG):
                nb = nch[t][g]
                if nb == 0:
                    continue
                si = meta.sig[g][t]
                nidx = nb * P
                G = gp.tile([P, max_nch * ELEM], BF16, tag=f"G{g}")
                nc.gpsimd.dma_gather(
                    out_ap=G[:, 0:nb * ELEM].rearrange(
                        "p (k d) -> p k d", d=ELEM),
                    in_ap=hs_full[g][:],
                    idxs_ap=idx_sb[:, si:si + nb * (P // 16)],
                    num_idxs=nidx, num_idxs_reg=nidx, elem_size=ELEM)
                st.setdefault("G", {})[g] = G

        def stage_eqd(t, prev_pe_ops):
            """eq build + per-chunk transpose/d-matmul, interleaved with the
            previous tile's aggregation matmul groups (prev_pe_ops)."""
            st = state[t]
            ntot = sum(nch[t])
            eqa = sbc.tile([P, max_ntot * P], BF16, tag="eqa")
            st["eqa"] = eqa
            dpsa = psd.tile([P, max(max_ntot * Hn, 4)], F32, tag="dpsa")
            st["dpsa"] = dpsa
            pe_iter = iter(prev_pe_ops)
            if ntot:
                nc.vector.memset(dpsa[:, 0:ntot * Hn], 0)
                c0 = meta.sc[t][0]
                dlv = dl_sb[:, c0:c0 + ntot].to_broadcast([P, ntot, P])
                io = iota_sb[:]
                iob = bass.AP(io.tensor, io.offset,
                              [list(io.ap[0]), [0, ntot], list(io.ap[1])])
                nc.vector.tensor_tensor(
                    out=eqa[:, 0:ntot * P].rearrange("p (k d) -> p k d", d=P),
                    in0=dlv, in1=iob, op=AL.is_equal)
                prev_eqT = None
                for k in range(ntot):
                    scr = pscr.tile([P, P], BF16, tag="scr")
                    nc.tensor.transpose(out=scr[:],
                                        in_=eqa[:, k * P:(k + 1) * P],
                                        identity=id_sb[:])
                    eqT = eqtp.tile([P, P], BF16, tag="eqT")
                    if k % 2 == 0:
                        nc.vector.tensor_copy(eqT[:], scr[:])
                    else:
                        nc.scalar.copy(eqT[:], scr[:])
                    ops = next(pe_iter, None)
                    if ops:
                        for f in ops:
                            f()
                    nc.tensor.matmul(
                        out=dpsa[:, k * Hn:(k + 1) * Hn],
                        lhsT=eqT[:], rhs=d_sb[:, t * Hn:(t + 1) * Hn],
                        start=False, stop=True, skip_group_check=True)
            for rest in pe_iter:
                for f in rest:
                    f()

        def stage_w(t):
            """tsd -> leaky -> exp -> wfb -> S builds; returns the per-chunk
            aggregation matmul closures for interleaving."""
            st = state[t]
            ntot = sum(nch[t])
            agg = psA.tile([P, CO + (1 if merge_den else 0)], F32, tag="agg")
            st["agg"] = agg
            if merge_den:
                den_ap = agg[:, CO:CO + 1]
            else:
                den_t = ps1.tile([P, Hn], F32, tag="den")
                den_ap = den_t[:]
            st["den_ap"] = den_ap
            if ntot == 0:
                nc.vector.memset(agg[:], 0)
                if not merge_den:
                    nc.vector.memset(den_t[:], 0)
                return []
            eqa = st["eqa"]
            dpsa = st["dpsa"]
            Gs = st["G"]
            tsda = sbc.tile([P, max_ntot * Hn], F32, tag="tsda")
            b0 = 0
            for g in range(c.NG):
                n = nch[t][g]
                if n == 0:
                    continue
                Gv = Gs[g][:, 0:n * ELEM].rearrange(
                    "p (k d) -> p k d", d=ELEM)[:, :, CO:CO + Hn]
                Dv = dpsa[:, b0 * Hn:(b0 + n) * Hn].rearrange(
                    "p (k h) -> p k h", h=Hn)
                nc.vector.tensor_tensor(
                    out=tsda[:, b0 * Hn:(b0 + n) * Hn].rearrange(
                        "p (k h) -> p k h", h=Hn),
                    in0=Gv, in1=Dv, op=AL.add)
                b0 += n
            lra = sbc.tile([P, max_ntot * Hn], F32, tag="lra")
            nc.vector.scalar_tensor_tensor(
                out=lra[:, 0:ntot * Hn], in0=tsda[:, 0:ntot * Hn],
                scalar=NEG_SLOPE_ATT, in1=tsda[:, 0:ntot * Hn],
                op0=AL.mult, op1=AL.max)
            wfa = sbc.tile([P, max_ntot * Hn], F32, tag="wfa")
            nc.scalar.activation(wfa[:, 0:ntot * Hn], lra[:, 0:ntot * Hn],
                                 AF.Exp)
            wfb = sbc.tile([P, max_ntot * Hn], BF16, tag="wfb")
            nc.vector.tensor_copy(wfb[:, 0:ntot * Hn], wfa[:, 0:ntot * Hn])

            pe_ops = []
            gchunk = 0
            for g in range(c.NG):
                n = nch[t][g]
                if n == 0:
                    continue
                G = Gs[g]
                for b in range(n):
                    k = gchunk
                    first, last = (k == 0), (k == ntot - 1)
                    wf = wfa[:, k * Hn:(k + 1) * Hn]
                    wb = wfb[:, k * Hn:(k + 1) * Hn]
                    eq = eqa[:, k * P:(k + 1) * P]
                    S = sSp.tile([P, Hn * P], BF16, tag="S")
                    eq_b = bass.AP(eq.tensor, eq.offset,
                                   [list(eq.ap[0]), [0, Hn], list(eq.ap[1])])
                    nc.vector.tensor_tensor(
                        out=S[:].rearrange("p (h d) -> p h d", h=Hn),
                        in0=eq_b, in1=wf.to_broadcast([P, Hn, P]),
                        op=AL.mult)

                    def mk(S=S, G=G, b=b, eq=eq, wb=wb, first=first,
                           last=last):
                        def run():
                            BK = 512
                            for h in range(Hn):
                                h_first = (h * Ch) % BK == 0
                                h_last = ((h + 1) * Ch) % BK == 0 or (
                                    h == Hn - 1 and not merge_den)
                                nc.tensor.matmul(
                                    out=agg[:, h * Ch:(h + 1) * Ch],
                                    lhsT=S[:, h * P:(h + 1) * P],
                                    rhs=G[:, b * ELEM + h * Ch:
                                            b * ELEM + (h + 1) * Ch],
                                    start=first and h_first,
                                    stop=last and h_last)
                            if merge_den:
                                nc.tensor.matmul(out=den_ap, lhsT=eq,
                                                 rhs=wb[:, 0:1], start=False,
                                                 stop=last)
                            else:
                                nc.tensor.matmul(out=den_ap, lhsT=eq, rhs=wb,
                                                 start=first, stop=last)
                        return run

                    pe_ops.append([mk()])
                    gchunk += 1
            return pe_ops

        # pipeline: gather(t+GD) | eqd(t+1) x agg(t) | w(t+1) | epilogue(t)
        GD = 3
        for t0 in range(min(GD, c.T)):
            stage_gather(t0)
        stage_eqd(0, [])
        pend = stage_w(0)
        for t in range(c.T):
            if t + GD < c.T:
                stage_gather(t + GD)
            if t + 1 < c.T:
                stage_eqd(t + 1, pend)
                pend = stage_w(t + 1)
            else:
                stage_eqd_flush(pend)
            _epilogue(nc, sb, ps1, c, meta, lay, t, state[t]["agg"],
                      state[t]["den_ap"], Hn, Ch, CO, bln_sb, id_sb,
                      fuse, final, y)
            del state[t]
            if ag_fire is not None:
                for b in range(c.NG):
                    if t == c.SBT[b] + c.BLK_T[b] - 1:
                        ag_fire(b)


def stage_eqd_flush(pend):
    for ops in pend:
        for f in ops:
            f()


def _epilogue(nc, sb, ps1, c, meta, lay, t, agg, den_ap, Hn, Ch, CO,
              bln_sb, id_sb, fuse, final, y):
    denr = sb.tile([P, Hn], F32, tag="denr")
    nc.vector.tensor_scalar(out=denr[:], in0=den_ap, scalar1=1e-16,
                            scalar2=None, op0=AL.add)
    rec = sb.tile([P, Hn], F32, tag="rec")
    nc.vector.reciprocal(rec[:], denr[:])
    ob = sb.tile([P, CO], F32 if final else BF16, tag="ob")
    for h in range(Hn):
        nc.vector.scalar_tensor_tensor(
            out=ob[:, h * Ch:(h + 1) * Ch], in0=agg[:, h * Ch:(h + 1) * Ch],
            scalar=rec[:, h:h + 1], op0=AL.mult,
            in1=bln_sb[:, h * Ch:(h + 1) * Ch], op1=AL.add)
    rs = sb.tile([P, 1], F32, tag="rs")
    nc.vector.tensor_reduce(out=rs[:], in_=ob[:], axis=mybir.AxisListType.X,
                            op=AL.add)
    nm = sb.tile([P, 1], F32, tag="nm")
    nc.vector.tensor_scalar(out=nm[:], in0=rs[:], scalar1=-1.0 / CO,
                            scalar2=None, op0=AL.mult)
    sqd = sb.tile([P, CO], BF16, tag="sqd")
    vs = sb.tile([P, 1], F32, tag="vs")
    nc.scalar.activation(sqd[:], ob[:], AF.Square, bias=nm[:, 0:1],
                         accum_out=vs[:])
    vstd = sb.tile([P, 1], F32, tag="vstd")
    nc.vector.tensor_scalar(out=vstd[:], in0=vs[:], scalar1=1.0 / CO,
                            scalar2=LN_EPS, op0=AL.mult, op1=AL.add)
    sd = sb.tile([P, 1], F32, tag="sd")
    nc.scalar.activation(sd[:], vstd[:], AF.Sqrt)
    rstd = sb.tile([P, 1], F32, tag="rstd")
    nc.vector.reciprocal(rstd[:], sd[:])
    xcs = sb.tile([P, CO], F32 if final else BF16, tag="xcs")
    nc.vector.tensor_scalar(out=xcs[:], in0=ob[:], scalar1=nm[:, 0:1],
                            scalar2=rstd[:, 0:1], op0=AL.add, op1=AL.mult)
    y1 = sb.tile([P, CO], F32 if final else BF16, tag="y1")
    nc.vector.tensor_tensor(out=y1[:], in0=xcs[:], in1=bln_sb[:, CO:2 * CO],
                            op=AL.mult)
    y2 = sb.tile([P, CO], F32 if final else BF16, tag="y2")
    nc.vector.tensor_tensor(out=y2[:], in0=y1[:], in1=bln_sb[:, 2 * CO:3 * CO],
                            op=AL.add)

    if final:
        mx = sb.tile([P, 1], F32, tag="mx")
        nc.vector.tensor_reduce(out=mx[:], in_=y2[:],
                                axis=mybir.AxisListType.X, op=AL.max)
        nmx = sb.tile([P, 1], F32, tag="nmx")
        nc.vector.tensor_scalar(out=nmx[:], in0=mx[:], scalar1=-1.0,
                                scalar2=None, op0=AL.mult)
        xs = sb.tile([P, CO], F32, tag="xs")
        nc.vector.tensor_scalar(out=xs[:], in0=y2[:], scalar1=nmx[:, 0:1],
                                scalar2=None, op0=AL.add)
        ex = sb.tile([P, CO], F32, tag="ex")
        se = sb.tile([P, 1], F32, tag="se")
        nc.scalar.activation(ex[:], xs[:], AF.Exp, accum_out=se[:])
        lse = sb.tile([P, 1], F32, tag="lse")
        nc.scalar.activation(lse[:], se[:], AF.Ln)
        nlse = sb.tile([P, 1], F32, tag="nlse")
        nc.vector.tensor_scalar(out=nlse[:], in0=lse[:], scalar1=-1.0,
                                scalar2=None, op0=AL.mult)
        yo = sb.tile([P, CO], F32, tag="yo")
        nc.vector.tensor_scalar(out=yo[:], in0=xs[:], scalar1=nlse[:, 0:1],
                                scalar2=None, op0=AL.add)
        nc.sync.dma_start(y[t * P:(t + 1) * P, :], yo[:])
        return

    x2 = sb.tile([P, CO], BF16, tag="x2")
    nc.vector.scalar_tensor_tensor(
        out=x2[:], in0=y2[:], scalar=NEG_SLOPE_ACT, in1=y2[:],
        op0=AL.mult, op1=AL.max)
    W_sb, KC, Ww = fuse["W_sb"], fuse["KC"], fuse["Ww"]
    CO2, Hn2, ELEMn = fuse["CO"], fuse["Hn2"], fuse["ELEMn"]
    xt2 = sb.tile([P, KC * P], BF16, tag="xt2")
    for k in range(KC):
        scr = ps1.tile([P, P], BF16, tag="scr")
        nc.tensor.transpose(out=scr[:], in_=x2[:, k * P:(k + 1) * P],
                            identity=id_sb[:])
        nc.scalar.copy(xt2[:, k * P:(k + 1) * P], scr[:])
    hp = ps1.tile([P, Ww], F32, tag="hnext")
    _mm_splits(nc, hp, xt2, W_sb, KC, Ww, P)
    _store_hs(nc, sb, hp[:, 0:CO2], hp[:, CO2:CO2 + Hn2],
              hp[:, CO2 + Hn2:CO2 + 2 * Hn2], CO2, Hn2, ELEMn,
              fuse["hs_loc"], fuse["d_next"], t)


# --------------------------------------------------------------------------
# entry point
# --------------------------------------------------------------------------

_CACHE = {}


def _get_nc(cfg, meta):
    key = (tuple(sorted((k, str(v)) for k, v in cfg.__dict__.items())),
           tuple(tuple(r) for r in meta.nch))
    if key not in _CACHE:
        _CACHE[key] = build_nc(cfg, meta)
    return _CACHE[key]


def kernel(**inputs):
    inputs = {k: np.asarray(v) for k, v in inputs.items()}
    x = inputs["x"]
    cfg = Cfg(N=x.shape[0], E=inputs["edge_src"].shape[0], F_IN=x.shape[1],
              HEADS=inputs["a_src1"].shape[0], C1=inputs["a_src1"].shape[1],
              C2=inputs["a_src2"].shape[1], NCLS=inputs["W3"].shape[0],
              NCORES=8)
    in_maps, meta = host_prep(cfg, **inputs)
    nc = _get_nc(cfg, meta)
    trace = bool(int(os.environ.get("GAT_TRACE", "0")))
    res = run_bass_kernel_spmd(nc, in_maps, core_ids=list(range(cfg.NCORES)),
                               trace=trace)
    global LAST_EXEC_NS, LAST_RES
    LAST_EXEC_NS = res.exec_time_ns
    LAST_RES = res
    out = np.concatenate(
        [res.results[cc]["y"][:cfg.NL] for cc in range(cfg.NCORES)], axis=0)
    return out.astype(np.float32)


LAST_EXEC_NS = None
LAST_RES = None


if __name__ == "__main__":
    pass



# revision 52
# speedup vs baseline: 1.0225x; 1.0225x over previous
"""3-layer GAT on 8 Trainium2 NeuronCores (Bass/Tile) — v4.

Pool-engine-lean design. The SWDGE dma_gather generation is the scarce
resource (~3.4us fixed/call, <=~1000 indices/call HW limit):
  - nodes block-mapped into NG=2 row blocks (int16 gather halves); ONE idx
    table serves the x-gather (layer 1) and the hs-table gathers (2/3).
  - one gather per (tile, block): ~98 calls/layer, nothing else on Pool.
  - layer-1 h/s come out of the per-edge transform of gathered raw x rows
    (512B/row instead of 2304B feature rows; no feature AllGather at all).
  - per-edge dst logits via PE transpose of the eq selection matrix + a
    tiny matmul against persistent SBUF d tables (written directly by the
    epilogues): no d gathers, no DRAM d tables.
  - layer-2/3 feature tables all-gathered per block (2 collectives each,
    RDH ~100-200GB/s), firing as the producing half completes.
  - emission is software-pipelined so the PE interleaves eq transposes
    with the previous tile's aggregation matmuls.

Self-contained: only imports the system concourse install.
"""

import os
import sys

for _p in ("/opt/trn_rl_repo", "/root/.axon_site/_ro/trn_rl_repo"):
    if os.path.isdir(_p) and _p not in sys.path:
        sys.path.insert(0, _p)

from dataclasses import dataclass

import ml_dtypes
import numpy as np

import concourse.bacc as bacc
import concourse.bass as bass
import concourse.tile as tile
from concourse import mybir
from concourse.bass_utils import run_bass_kernel_spmd

P = 128
BF16 = mybir.dt.bfloat16
F32 = mybir.dt.float32
I16 = mybir.dt.int16
AL = mybir.AluOpType
AF = mybir.ActivationFunctionType

NEG_SLOPE_ATT = 0.2
NEG_SLOPE_ACT = 0.01
LN_EPS = 1e-5
MAX_GIDX = 896           # dma_gather HW limit: <=~1000 indices per call


def _ceil(a, b):
    return -(-a // b)


def _pad_elem(n_f32_elems):
    """bf16 row length (elements) padded so row bytes are a multiple of 256."""
    return _ceil(n_f32_elems * 2, 256) * 128


@dataclass
class Cfg:
    N: int = 50000
    E: int = 400000
    F_IN: int = 256
    HEADS: int = 4
    C1: int = 256
    C2: int = 128
    NCLS: int = 32
    NCORES: int = 8

    def __post_init__(self):
        assert self.N % self.NCORES == 0
        self.NL = self.N // self.NCORES
        self.T = _ceil(self.NL, P)
        self.NLP = self.T * P
        self.NPTOT = self.NLP * self.NCORES
        self.NG = 2
        # asymmetric: block 0's (large) AllGather fires mid-L1 and hides
        # behind the remaining tiles; only block 1's smaller AG sits at
        # the L1->L2 boundary.
        self.BLK_T = [29, 20]
        assert sum(self.BLK_T) == self.T
        self.SBT = [0, 29]
        self.GROWS = [b * P * self.NCORES for b in self.BLK_T]
        self.GBASE = [0, self.GROWS[0]]
        assert max(self.GROWS) < 32768  # int16 gather indices per group
        H = self.HEADS
        self.CO1 = H * self.C1
        self.CO2 = H * self.C2
        assert self.F_IN % P == 0 and self.CO1 % P == 0 and self.CO2 % P == 0
        self.ELEM2 = _pad_elem(self.CO2 + H)          # [h2|s2|pad] rows
        self.ELEM3 = _pad_elem(self.NCLS + 1)         # [h3|s3|pad] rows
        self.W1w = self.CO1 + 2 * H                   # [W1 | U_s | U_d]
        self.W2w = self.CO2 + 2 * H
        self.W3w = self.NCLS + 2


@dataclass
class Meta:
    nch: list   # [T][NG] chunk counts (common across cores)
    sig: list   # [NG][T] idx col offsets (group-major)
    sc: list    # [T][NG] dstloc column offsets (tile-major)
    SI: int
    SC: int


def _grp_map(cfg: Cfg, core, loc):
    """(source block, within-block row index) for node (core, local idx)."""
    t = loc // P
    b = (t >= cfg.BLK_T[0]).astype(np.int64)
    blk_t = np.array(cfg.BLK_T)[b]
    sb = np.array(cfg.SBT)[b]
    return b, core * blk_t * P + (loc - sb * P)


def _gidx_map(cfg: Cfg, core, loc):
    b, off = _grp_map(cfg, core, loc)
    return np.array(cfg.GBASE)[b] + off


def host_prep(cfg: Cfg, x, edge_src, edge_dst,
              W1, a_src1, a_dst1, b1, ln1_g, ln1_b,
              W2, a_src2, a_dst2, b2, ln2_g, ln2_b,
              W3, a_src3, a_dst3, b3, ln3_g, ln3_b):
    c = cfg
    bf = ml_dtypes.bfloat16

    # ---- append self loops, shard edges by destination core
    loops = np.arange(c.N, dtype=np.int64)
    src = np.concatenate([edge_src.astype(np.int64), loops])
    dst = np.concatenate([edge_dst.astype(np.int64), loops])

    dst_core = dst // c.NL
    dstloc = dst - dst_core * c.NL
    tile_id = dstloc // P
    grp, idx16 = _grp_map(c, src // c.NL, src % c.NL)
    grp = grp.astype(np.int64)
    idx16 = idx16.astype(np.int64)
    NG = c.NG

    counts = np.zeros((c.NCORES, c.T, NG), np.int64)
    np.add.at(counts, (dst_core, tile_id, grp), 1)
    nch = np.maximum(_ceil(counts.max(axis=0), P), 0)  # [T,NG] chunks
    assert nch.max() * P <= MAX_GIDX, nch.max()
    sig = np.zeros((NG, c.T), np.int64)
    acc = 0
    for g in range(NG):
        for t in range(c.T):
            sig[g, t] = acc
            acc += int(nch[t, g]) * (P // 16)
    SI = int(acc)
    sc = np.zeros((c.T, NG), np.int64)
    acc_sc = 0
    for t in range(c.T):
        for g in range(NG):
            sc[t, g] = acc_sc
            acc_sc += int(nch[t, g])
    SC = int(acc_sc)
    meta = Meta(nch=nch.tolist(), sig=sig.tolist(), sc=sc.tolist(),
                SI=SI, SC=SC)

    order = np.lexsort((grp, tile_id, dst_core))
    src_s = idx16[order]
    dstrel_s = (dstloc - tile_id * P)[order]

    starts = np.zeros((c.NCORES, c.T, NG), np.int64)
    run = 0
    for cc in range(c.NCORES):
        for t in range(c.T):
            for g in range(NG):
                starts[cc, t, g] = run
                run += int(counts[cc, t, g])

    idx_tabs, dl_tabs = [], []
    for cc in range(c.NCORES):
        itab = np.zeros((16, SI), np.int16)
        dtab = np.full((P, SC), -1.0, np.float32)
        for t in range(c.T):
            for g in range(NG):
                m = int(counts[cc, t, g])
                n = int(nch[t, g])
                if n == 0:
                    continue
                s0 = int(starts[cc, t, g])
                iv = np.zeros(n * P, np.int16)
                iv[:m] = src_s[s0:s0 + m].astype(np.int16)
                cols = int(sig[g, t])
                itab[:, cols:cols + n * (P // 16)] = iv.reshape(
                    n * P // 16, 16).T
                dv = np.full(n * P, -1.0, np.float32)
                dv[:m] = dstrel_s[s0:s0 + m].astype(np.float32)
                dtab[:, sc[t, g]:sc[t, g] + n] = dv.reshape(n, P).T
        idx_tabs.append(np.tile(itab, (8, 1)))
        dl_tabs.append(dtab)

    # ---- block-mapped full x table (replicated to every core)
    xfull = np.zeros((c.NPTOT, c.F_IN), np.float32)
    for cc in range(c.NCORES):
        loc = np.arange(c.NL)
        gi = _gidx_map(c, np.full(c.NL, cc), loc)
        xfull[gi] = x[cc * c.NL:(cc + 1) * c.NL]
    xfull = xfull.astype(bf)

    # ---- weights (augmented with U = W.T @ a columns), bf16
    def aug(W, a_s, a_d, H, C):
        WT = W.T.astype(np.float64)
        U_s = np.zeros((WT.shape[0], H))
        U_d = np.zeros((WT.shape[0], H))
        for h in range(H):
            U_s[:, h] = WT[:, h * C:(h + 1) * C] @ a_s[h].astype(np.float64)
            U_d[:, h] = WT[:, h * C:(h + 1) * C] @ a_d[h].astype(np.float64)
        return np.concatenate([WT, U_s, U_d], axis=1).astype(bf)

    W1a = aug(W1, a_src1, a_dst1, c.HEADS, c.C1)   # [F_IN, CO1+2H]
    W2a = aug(W2, a_src2, a_dst2, c.HEADS, c.C2)   # [CO1, CO2+2H]
    W3a = aug(W3, a_src3, a_dst3, 1, c.NCLS)       # [CO2, NCLS+2]

    def bln(b, g, be):
        row = np.concatenate([b, g, be]).astype(np.float32)[None, :]
        return np.repeat(row, P, axis=0)

    bln1 = bln(b1, ln1_g, ln1_b)
    bln2 = bln(b2, ln2_g, ln2_b)
    bln3 = bln(b3, ln3_g, ln3_b)

    ident = np.eye(P, dtype=bf)
    iota_f = np.repeat(np.arange(P, dtype=np.float32)[None, :], P, axis=0)

    in_maps = []
    for cc in range(c.NCORES):
        xl = np.zeros((c.NLP, c.F_IN), np.float32)
        xl[:c.NL] = x[cc * c.NL:(cc + 1) * c.NL]
        in_maps.append({
            "xT": np.ascontiguousarray(xl.T).astype(bf),
            "xfull": xfull,
            "W1a": W1a, "W2a": W2a, "W3a": W3a,
            "bln1": bln1, "bln2": bln2, "bln3": bln3,
            "idx16": idx_tabs[cc], "dstloc": dl_tabs[cc],
            "iotaf": iota_f, "ident": ident,
        })
    return in_maps, meta


# --------------------------------------------------------------------------
# device program
# --------------------------------------------------------------------------

def build_nc(cfg: Cfg, meta: Meta):
    c = cfg
    H = c.HEADS
    nc = bacc.Bacc("TRN2", target_bir_lowering=False, debug=False,
                   num_devices=c.NCORES, enable_partition_id=False)

    # ---- I/O
    xT = nc.dram_tensor("xT", [c.F_IN, c.NLP], BF16, kind="ExternalInput").ap()
    xfull = nc.dram_tensor("xfull", [c.NPTOT, c.F_IN], BF16,
                           kind="ExternalInput").ap()
    W1a = nc.dram_tensor("W1a", [c.F_IN, c.W1w], BF16, kind="ExternalInput").ap()
    W2a = nc.dram_tensor("W2a", [c.CO1, c.W2w], BF16, kind="ExternalInput").ap()
    W3a = nc.dram_tensor("W3a", [c.CO2, c.W3w], BF16, kind="ExternalInput").ap()
    bln1 = nc.dram_tensor("bln1", [P, 3 * c.CO1], F32, kind="ExternalInput").ap()
    bln2 = nc.dram_tensor("bln2", [P, 3 * c.CO2], F32, kind="ExternalInput").ap()
    bln3 = nc.dram_tensor("bln3", [P, 3 * c.NCLS], F32, kind="ExternalInput").ap()
    idx16 = nc.dram_tensor("idx16", [P, meta.SI], I16, kind="ExternalInput").ap()
    dstloc = nc.dram_tensor("dstloc", [P, meta.SC], F32, kind="ExternalInput").ap()
    iotaf = nc.dram_tensor("iotaf", [P, P], F32, kind="ExternalInput").ap()
    ident = nc.dram_tensor("ident", [P, P], BF16, kind="ExternalInput").ap()
    y = nc.dram_tensor("y", [c.NLP, c.NCLS], F32, kind="ExternalOutput").ap()

    groups = [list(range(c.NCORES))]

    def block_ag(loc_t, full_ts):
        for b in range(c.NG):
            r0 = c.SBT[b] * P
            nr = c.BLK_T[b] * P
            nc.gpsimd.collective_compute(
                "AllGather", AL.bypass, replica_groups=groups,
                ins=[loc_t[r0:r0 + nr, :].opt()],
                outs=[full_ts[b][:].opt()])

    with tile.TileContext(nc) as tc:
        dram_cm = tc.tile_pool(name="dram", bufs=1, space="DRAM")
        dram = dram_cm.__enter__()
        hs2_loc = dram.tile([c.NLP, c.ELEM2], BF16)
        hs2_full = [dram.tile([c.GROWS[b], c.ELEM2], BF16, addr_space="Shared",
                              name=f"hs2f{b}") for b in range(c.NG)]
        hs3_loc = dram.tile([c.NLP, c.ELEM3], BF16)
        hs3_full = [dram.tile([c.GROWS[b], c.ELEM3], BF16, addr_space="Shared",
                              name=f"hs3f{b}") for b in range(c.NG)]

        # ---- persistent SBUF constants
        cpool_cm = tc.tile_pool(name="const", bufs=1)
        cpool = cpool_cm.__enter__()
        KC1 = c.F_IN // P
        W1a_sb = cpool.tile([P, KC1 * c.W1w], BF16)
        for k in range(KC1):
            nc.sync.dma_start(W1a_sb[:, k * c.W1w:(k + 1) * c.W1w],
                              W1a[k * P:(k + 1) * P, :])
        KC2 = c.CO1 // P
        W2a_sb = cpool.tile([P, KC2 * c.W2w], BF16)
        for k in range(KC2):
            nc.sync.dma_start(W2a_sb[:, k * c.W2w:(k + 1) * c.W2w],
                              W2a[k * P:(k + 1) * P, :])
        KC3 = c.CO2 // P
        W3a_sb = cpool.tile([P, KC3 * c.W3w], BF16)
        for k in range(KC3):
            nc.sync.dma_start(W3a_sb[:, k * c.W3w:(k + 1) * c.W3w],
                              W3a[k * P:(k + 1) * P, :])
        bln1_sb = cpool.tile([P, 3 * c.CO1], F32)
        nc.sync.dma_start(bln1_sb[:], bln1[:])
        bln2_sb = cpool.tile([P, 3 * c.CO2], F32)
        nc.sync.dma_start(bln2_sb[:], bln2[:])
        bln3_sb = cpool.tile([P, 3 * c.NCLS], F32)
        nc.sync.dma_start(bln3_sb[:], bln3[:])
        idx_sb = cpool.tile([P, meta.SI], I16)
        nc.sync.dma_start(idx_sb[:], idx16[:])
        dl_sb = cpool.tile([P, meta.SC], F32)
        nc.sync.dma_start(dl_sb[:], dstloc[:])
        iota_sb = cpool.tile([P, P], F32)
        nc.sync.dma_start(iota_sb[:], iotaf[:])
        id_sb = cpool.tile([P, P], BF16)
        nc.sync.dma_start(id_sb[:], ident[:])
        # persistent per-layer local d tables (dst logits), bf16
        d1_sb = cpool.tile([P, c.T * H], BF16)
        d2_sb = cpool.tile([P, c.T * H], BF16)
        d3_sb = cpool.tile([P, c.T * 1], BF16)

        # ================= prologue: local d1 logits -> d1_sb
        with (
            tc.tile_pool(name="pro", bufs=3) as pro,
            tc.tile_pool(name="prop", bufs=2, space="PSUM") as prop,
        ):
            for t in range(c.T):
                xt = pro.tile([P, KC1 * P], BF16, tag="xt")
                for k in range(KC1):
                    nc.sync.dma_start(xt[:, k * P:(k + 1) * P],
                                      xT[k * P:(k + 1) * P, t * P:(t + 1) * P])
                dsp = prop.tile([P, H], F32, tag="dsp")
                for k in range(KC1):
                    nc.tensor.matmul(
                        out=dsp[:],
                        lhsT=xt[:, k * P:(k + 1) * P],
                        rhs=W1a_sb[:, k * c.W1w + c.CO1 + H:
                                   k * c.W1w + c.CO1 + 2 * H],
                        start=(k == 0), stop=(k == KC1 - 1))
                nc.vector.tensor_copy(d1_sb[:, t * H:(t + 1) * H], dsp[:])

        def ag2_fire(b):
            r0 = c.SBT[b] * P
            nr = c.BLK_T[b] * P
            nc.gpsimd.collective_compute(
                "AllGather", AL.bypass, replica_groups=groups,
                ins=[hs2_loc[r0:r0 + nr, :].opt()],
                outs=[hs2_full[b][:].opt()])

        # ================= layer 1
        _l1_phase(nc, tc, c, meta, xfull, W1a_sb, W2a_sb, bln1_sb,
                  iota_sb, id_sb, idx_sb, dl_sb, d1_sb, d2_sb, hs2_loc,
                  ag2_fire)

        # ================= layer 2 (+fused L3 transform)
        _edge_phase(
            nc, tc, c, meta, lay=2, Hn=H, Ch=c.C2, ELEM=c.ELEM2,
            hs_full=hs2_full, d_sb=d2_sb, bln_sb=bln2_sb,
            iota_sb=iota_sb, id_sb=id_sb, idx_sb=idx_sb, dl_sb=dl_sb,
            fuse=dict(W_sb=W3a_sb, KC=KC3, Ww=c.W3w, CO=c.NCLS, Hn2=1,
                      ELEMn=c.ELEM3, hs_loc=hs3_loc, d_next=d3_sb),
            final=None, y=None)
        block_ag(hs3_loc, hs3_full)

        # ================= layer 3 + log_softmax
        _edge_phase(
            nc, tc, c, meta, lay=3, Hn=1, Ch=c.NCLS, ELEM=c.ELEM3,
            hs_full=hs3_full, d_sb=d3_sb, bln_sb=bln3_sb,
            iota_sb=iota_sb, id_sb=id_sb, idx_sb=idx_sb, dl_sb=dl_sb,
            fuse=None, final=True, y=y)

        cpool_cm.__exit__(None, None, None)
        dram_cm.__exit__(None, None, None)

    nc.compile()
    return nc


# --------------------------------------------------------------------------
# layer-1 phase: per-edge transform fused with aggregation
# --------------------------------------------------------------------------

def _l1_phase(nc, tc, c: Cfg, meta: Meta, xfull, W1a_sb, W2a_sb, bln_sb,
              iota_sb, id_sb, idx_sb, dl_sb, d1_sb, d2_sb, hs2_loc,
              ag_fire):
    H = c.HEADS
    CO = c.CO1
    Ch = c.C1
    nch = meta.nch
    max_ntot = max(sum(r) for r in nch)
    max_nch = max(max(r) for r in nch)
    KC1 = c.F_IN // P
    GH_ACT = 768          # PSUM->SBUF copy split: ACT cols, DVE the rest
    DOFF = H + max_ntot * H   # d section start in the sden bank

    with (
        tc.tile_pool(name="gx", bufs=9) as gxp,
        tc.tile_pool(name="gh", bufs=max_ntot + 4) as ghp,
        tc.tile_pool(name="w1", bufs=2) as wp,
        tc.tile_pool(name="s1", bufs=3) as sp,
        tc.tile_pool(name="eqt", bufs=2) as eqtp,
        tc.tile_pool(name="ep", bufs=1) as ep,
        tc.tile_pool(name="x2p", bufs=3) as x2p,
        tc.tile_pool(name="hst1", bufs=2) as hstp,
        tc.tile_pool(name="php", bufs=3, space="PSUM") as php,
        tc.tile_pool(name="pagg", bufs=1, space="PSUM") as pagg,
        tc.tile_pool(name="psd", bufs=2, space="PSUM") as psd,
        tc.tile_pool(name="pscr", bufs=1, space="PSUM") as pscr,
    ):
        state = {}

        def stage_gather(t):
            st = state.setdefault(t, {})
            for g in range(c.NG):
                nb = nch[t][g]
                if nb == 0:
                    continue
                si = meta.sig[g][t]
                nidx = nb * P
                Gx = gxp.tile([P, KC1 * max_nch * P], BF16, tag=f"Gx{g}")
                nc.gpsimd.dma_gather(
                    out_ap=Gx[:, 0:KC1 * nidx].rearrange(
                        "p (j e) -> p j e", e=nidx),
                    in_ap=xfull[c.GBASE[g]:c.GBASE[g] + c.GROWS[g], :],
                    idxs_ap=idx_sb[:, si:si + nb * (P // 16)],
                    num_idxs=nidx, num_idxs_reg=nidx, elem_size=c.F_IN,
                    transpose=True)
                st.setdefault("Gx", {})[g] = Gx

        def stage_transform(t):
            """eq build; per chunk: transform h + s, interleaved with the
            eq transpose + d matmul of the previous chunk."""
            st = state[t]
            ntot = sum(nch[t])
            sden = psd.tile([P, 512], F32, tag="sden")
            st["sden"] = sden
            nc.vector.memset(sden[:, 0:DOFF + max_ntot * H], 0)
            eqa = wp.tile([P, max_ntot * P], BF16, tag="eqa")
            st["eqa"] = eqa
            if ntot == 0:
                return
            c0 = meta.sc[t][0]
            dlv = dl_sb[:, c0:c0 + ntot].to_broadcast([P, ntot, P])
            io = iota_sb[:]
            iob = bass.AP(io.tensor, io.offset,
                          [list(io.ap[0]), [0, ntot], list(io.ap[1])])
            nc.vector.tensor_tensor(
                out=eqa[:, 0:ntot * P].rearrange("p (k d) -> p k d", d=P),
                in0=dlv, in1=iob, op=AL.is_equal)

            eqTs = {}

            def trans(k):
                scr = pscr.tile([P, P], BF16, tag="scr")
                nc.tensor.transpose(out=scr[:],
                                    in_=eqa[:, k * P:(k + 1) * P],
                                    identity=id_sb[:])
                eqT = eqtp.tile([P, P], BF16, tag="eqT")
                # alternate the PSUM->SBUF copy between DVE and ACT so
                # neither engine eats the whole ~1470-copy bill
                if k % 3 == 0:
                    nc.vector.tensor_copy(eqT[:], scr[:])
                else:
                    nc.scalar.copy(eqT[:], scr[:])
                eqTs[k] = eqT

            def dmm(k):
                nc.tensor.matmul(
                    out=sden[:, DOFF + k * H:DOFF + (k + 1) * H],
                    lhsT=eqTs.pop(k)[:], rhs=d1_sb[:, t * H:(t + 1) * H],
                    start=False, stop=True, skip_group_check=True)

            Ghs = []
            ci = 0
            for g in range(c.NG):
                if nch[t][g] == 0:
                    continue
                Gx = st["Gx"][g]
                nb = nch[t][g]
                Gxv = Gx[:, 0:KC1 * nb * P].rearrange(
                    "p (j e) -> p j e", e=nb * P)
                for ck in range(nb):
                    Gh = ghp.tile([P, CO], BF16, tag="gh")
                    for half in range(2):
                        hp = php.tile([P, 512], F32, tag="hp")
                        n0 = half * 512
                        for j in range(KC1):
                            nc.tensor.matmul(
                                out=hp[:],
                                lhsT=Gxv[:, j, ck * P:(ck + 1) * P],
                                rhs=W1a_sb[:, j * c.W1w + n0:
                                           j * c.W1w + n0 + 512],
                                start=(j == 0), stop=(j == KC1 - 1))
                        if half == 0:
                            nc.scalar.copy(Gh[:, 0:512], hp[:])
                        else:
                            nc.scalar.copy(Gh[:, 512:GH_ACT],
                                           hp[:, 0:GH_ACT - 512])
                            nc.vector.tensor_copy(Gh[:, GH_ACT:CO],
                                                  hp[:, GH_ACT - 512:512])
                    for j in range(KC1):
                        nc.tensor.matmul(
                            out=sden[:, H + ci * H:H + (ci + 1) * H],
                            lhsT=Gxv[:, j, ck * P:(ck + 1) * P],
                            rhs=W1a_sb[:, j * c.W1w + CO:j * c.W1w + CO + H],
                            start=False, stop=(j == KC1 - 1),
                            skip_group_check=True)
                    trans(ci)
                    if ci >= 1:
                        dmm(ci - 1)
                    Ghs.append(Gh)
                    ci += 1
            dmm(ntot - 1)
            st["Gh"] = Ghs

        def stage_w_agg(t):
            st = state[t]
            ntot = sum(nch[t])
            agg = pagg.tile([P, CO], F32, tag="agg")
            st["agg"] = agg
            sden = st["sden"]
            st["den_ap"] = sden[:, 0:H]
            if ntot == 0:
                nc.vector.memset(agg[:], 0)
                return
            eqa = st["eqa"]
            ssb = wp.tile([P, max_ntot * H], F32, tag="ssb")
            nc.vector.tensor_copy(ssb[:, 0:ntot * H], sden[:, H:H + ntot * H])
            tsda = wp.tile([P, max_ntot * H], F32, tag="tsda")
            nc.vector.tensor_tensor(
                out=tsda[:, 0:ntot * H],
                in0=ssb[:, 0:ntot * H],
                in1=sden[:, DOFF:DOFF + ntot * H], op=AL.add)
            lra = wp.tile([P, max_ntot * H], F32, tag="lra")
            nc.vector.scalar_tensor_tensor(
                out=lra[:, 0:ntot * H], in0=tsda[:, 0:ntot * H],
                scalar=NEG_SLOPE_ATT, in1=tsda[:, 0:ntot * H],
                op0=AL.mult, op1=AL.max)
            wfa = wp.tile([P, max_ntot * H], F32, tag="wfa")
            nc.scalar.activation(wfa[:, 0:ntot * H], lra[:, 0:ntot * H], AF.Exp)
            wfb = wp.tile([P, max_ntot * H], BF16, tag="wfb")
            nc.vector.tensor_copy(wfb[:, 0:ntot * H], wfa[:, 0:ntot * H])

            for k in range(ntot):
                first, last = (k == 0), (k == ntot - 1)
                eq = eqa[:, k * P:(k + 1) * P]
                wf = wfa[:, k * H:(k + 1) * H]
                S = sp.tile([P, H * P], BF16, tag="S")
                eq_b = bass.AP(eq.tensor, eq.offset,
                               [list(eq.ap[0]), [0, H], list(eq.ap[1])])
                nc.vector.tensor_tensor(
                    out=S[:].rearrange("p (h d) -> p h d", h=H),
                    in0=eq_b, in1=wf.to_broadcast([P, H, P]), op=AL.mult)
                Gh = st["Gh"][k]
                for h in range(H):
                    nc.tensor.matmul(
                        out=agg[:, h * Ch:(h + 1) * Ch],
                        lhsT=S[:, h * P:(h + 1) * P],
                        rhs=Gh[:, h * Ch:(h + 1) * Ch],
                        start=first and (h * Ch) % 512 == 0,
                        stop=last and ((h + 1) * Ch) % 512 == 0)
                nc.tensor.matmul(out=sden[:, 0:H], lhsT=eq,
                                 rhs=wfb[:, k * H:(k + 1) * H],
                                 start=False, stop=last,
                                 skip_group_check=True)

        def stage_ln(t):
            st = state[t]
            agg, den_ap = st["agg"], st["den_ap"]
            denr = ep.tile([P, H], F32, tag="denr")
            nc.vector.tensor_scalar(out=denr[:], in0=den_ap, scalar1=1e-16,
                                    scalar2=None, op0=AL.add)
            rec = ep.tile([P, H], F32, tag="rec")
            nc.vector.reciprocal(rec[:], denr[:])
            ob = ep.tile([P, CO], BF16, tag="ob")
            for h in range(H):
                nc.vector.scalar_tensor_tensor(
                    out=ob[:, h * Ch:(h + 1) * Ch],
                    in0=agg[:, h * Ch:(h + 1) * Ch],
                    scalar=rec[:, h:h + 1], op0=AL.mult,
                    in1=bln_sb[:, h * Ch:(h + 1) * Ch], op1=AL.add)
            rs = ep.tile([P, 1], F32, tag="rs")
            nc.vector.tensor_reduce(out=rs[:], in_=ob[:],
                                    axis=mybir.AxisListType.X, op=AL.add)
            nm = ep.tile([P, 1], F32, tag="nm")
            nc.vector.tensor_scalar(out=nm[:], in0=rs[:], scalar1=-1.0 / CO,
                                    scalar2=None, op0=AL.mult)
            sqd = ep.tile([P, CO], BF16, tag="sqd")
            vs = ep.tile([P, 1], F32, tag="vs")
            nc.scalar.activation(sqd[:], ob[:], AF.Square, bias=nm[:, 0:1],
                                 accum_out=vs[:])
            vstd = ep.tile([P, 1], F32, tag="vstd")
            nc.vector.tensor_scalar(out=vstd[:], in0=vs[:], scalar1=1.0 / CO,
                                    scalar2=LN_EPS, op0=AL.mult, op1=AL.add)
            sd = ep.tile([P, 1], F32, tag="sd")
            nc.scalar.activation(sd[:], vstd[:], AF.Sqrt)
            rstd = ep.tile([P, 1], F32, tag="rstd")
            nc.vector.reciprocal(rstd[:], sd[:])
            xcs = ep.tile([P, CO], BF16, tag="xcs")
            nc.vector.tensor_scalar(out=xcs[:], in0=ob[:],
                                    scalar1=nm[:, 0:1], scalar2=rstd[:, 0:1],
                                    op0=AL.add, op1=AL.mult)
            y1 = ep.tile([P, CO], BF16, tag="y1")
            nc.vector.tensor_tensor(out=y1[:], in0=xcs[:],
                                    in1=bln_sb[:, CO:2 * CO], op=AL.mult)
            y2 = ep.tile([P, CO], BF16, tag="y2")
            nc.vector.tensor_tensor(out=y2[:], in0=y1[:],
                                    in1=bln_sb[:, 2 * CO:3 * CO], op=AL.add)
            x2 = x2p.tile([P, CO], BF16, tag="x2")
            nc.vector.scalar_tensor_tensor(
                out=x2[:], in0=y2[:], scalar=NEG_SLOPE_ACT, in1=y2[:],
                op0=AL.mult, op1=AL.max)
            st["x2"] = x2

        def stage_tail(t):
            st = state[t]
            x2 = st["x2"]
            KCn = CO // P
            xt2 = ep.tile([P, KCn * P], BF16, tag="xt2")
            for k in range(KCn):
                scr = pscr.tile([P, P], BF16, tag="scr")
                nc.tensor.transpose(out=scr[:], in_=x2[:, k * P:(k + 1) * P],
                                    identity=id_sb[:])
                nc.scalar.copy(xt2[:, k * P:(k + 1) * P], scr[:])
            hp = php.tile([P, 512], F32, tag="hp")
            hp2 = php.tile([P, 512], F32, tag="hp")
            _mm_splits(nc, hp[:], xt2, W2a_sb, KCn, c.W2w, P, upto=512)
            _mm_splits(nc, hp2[:, 0:c.W2w - 512], xt2, W2a_sb, KCn, c.W2w, P,
                       frm=512)
            _store_hs(nc, hstp, hp[:, 0:c.CO2], hp2[:, 0:H],
                      hp2[:, H:2 * H], c.CO2, c.HEADS, c.ELEM2,
                      hs2_loc, d2_sb, t)
            del state[t]

        def do_tail(tt):
            stage_tail(tt)
            for b in range(c.NG):
                if tt == c.SBT[b] + c.BLK_T[b] - 1:
                    ag_fire(b)

        GD = 6  # gather prefetch depth: rides out the block-0 AG stall
        for t0 in range(min(GD, c.T)):
            stage_gather(t0)
        for t in range(c.T):
            if t + GD < c.T:
                stage_gather(t + GD)
            stage_transform(t)
            stage_w_agg(t)
            stage_ln(t)
            if t - 2 >= 0:
                do_tail(t - 2)
        do_tail(c.T - 2)
        do_tail(c.T - 1)


def _mm_splits(nc, out_ps, lhs_sb, w_sb, KC, Ww, Plhs, frm=0, upto=None):
    """out_ps = sum_k lhs_k.T @ W_k[:, frm:upto], N split at 512."""
    upto = Ww if upto is None else min(upto, Ww)
    splits = []
    n0 = frm
    while n0 < upto:
        nsz = min(512, upto - n0)
        splits.append((n0, nsz))
        n0 += nsz
    for k in range(KC):
        for (n0, nsz) in splits:
            nc.tensor.matmul(
                out=out_ps[:, n0 - frm:n0 - frm + nsz],
                lhsT=lhs_sb[:, k * Plhs:(k + 1) * Plhs],
                rhs=w_sb[:, k * Ww + n0:k * Ww + n0 + nsz],
                start=(k == 0), stop=(k == KC - 1))


def _store_hs(nc, pool, h_ap, s_ap, d_ap, CO, Hn, ELEM, hs_loc, d_next_sb, t):
    """PSUM views -> bf16 [h|s|pad] row tile + SBUF d-table slice."""
    hst = pool.tile([P, ELEM], BF16, tag="hst")
    nc.scalar.copy(hst[:, 0:CO], h_ap)
    nc.vector.tensor_copy(hst[:, CO:CO + Hn], s_ap)
    if ELEM > CO + Hn:
        nc.vector.memset(hst[:, CO + Hn:ELEM], 0)
    nc.vector.tensor_copy(d_next_sb[:, t * Hn:(t + 1) * Hn], d_ap)
    nc.sync.dma_start(hs_loc[t * P:(t + 1) * P, :], hst[:])


# --------------------------------------------------------------------------
# layers 2/3: table-gather edge phase (tile-pipelined, eqT d logits)
# --------------------------------------------------------------------------

def _edge_phase(nc, tc, c: Cfg, meta: Meta, lay, Hn, Ch, ELEM, hs_full, d_sb,
                bln_sb, iota_sb, id_sb, idx_sb, dl_sb, fuse, final, y):
    CO = Hn * Ch
    nch = meta.nch
    max_ntot = max(sum(r) for r in nch)
    max_nch = max(max(r) for r in nch)
    merge_den = (Hn == 1)

    with (
        tc.tile_pool(name=f"sb{lay}", bufs=2) as sb,
        tc.tile_pool(name=f"sc{lay}", bufs=4) as sbc,
        tc.tile_pool(name=f"sS{lay}", bufs=max_ntot + 2) as sSp,
        tc.tile_pool(name=f"g{lay}", bufs=3) as gp,
        tc.tile_pool(name=f"eqt{lay}", bufs=3) as eqtp,
        tc.tile_pool(name=f"ps{lay}", bufs=1, space="PSUM") as ps1,
        tc.tile_pool(name=f"psagg{lay}", bufs=2, space="PSUM") as psA,
        tc.tile_pool(name=f"psd{lay}", bufs=2, space="PSUM") as psd,
        tc.tile_pool(name=f"pscr{lay}", bufs=1, space="PSUM") as pscr,
    ):
        state = {}

        def stage_gather(t):
            st = state.setdefault(t, {})
            for g in range(c.NG):
                nb = nch[t][g]
                if nb == 0:
                    continue
                si = meta.sig[g][t]
                nidx = nb * P
                G = gp.tile([P, max_nch * ELEM], BF16, tag=f"G{g}")
                nc.gpsimd.dma_gather(
                    out_ap=G[:, 0:nb * ELEM].rearrange(
                        "p (k d) -> p k d", d=ELEM),
                    in_ap=hs_full[g][:],
                    idxs_ap=idx_sb[:, si:si + nb * (P // 16)],
                    num_idxs=nidx, num_idxs_reg=nidx, elem_size=ELEM)
                st.setdefault("G", {})[g] = G

        def stage_eqd(t, prev_pe_ops):
            """eq build + per-chunk transpose/d-matmul, interleaved with the
            previous tile's aggregation matmul groups (prev_pe_ops)."""
            st = state[t]
            ntot = sum(nch[t])
            eqa = sbc.tile([P, max_ntot * P], BF16, tag="eqa")
            st["eqa"] = eqa
            dpsa = psd.tile([P, max(max_ntot * Hn, 4)], F32, tag="dpsa")
            st["dpsa"] = dpsa
            pe_iter = iter(prev_pe_ops)
            if ntot:
                nc.vector.memset(dpsa[:, 0:ntot * Hn], 0)
                c0 = meta.sc[t][0]
                dlv = dl_sb[:, c0:c0 + ntot].to_broadcast([P, ntot, P])
                io = iota_sb[:]
                iob = bass.AP(io.tensor, io.offset,
                              [list(io.ap[0]), [0, ntot], list(io.ap[1])])
                nc.vector.tensor_tensor(
                    out=eqa[:, 0:ntot * P].rearrange("p (k d) -> p k d", d=P),
                    in0=dlv, in1=iob, op=AL.is_equal)
                prev_eqT = None
                for k in range(ntot):
                    scr = pscr.tile([P, P], BF16, tag="scr")
                    nc.tensor.transpose(out=scr[:],
                                        in_=eqa[:, k * P:(k + 1) * P],
                                        identity=id_sb[:])
                    eqT = eqtp.tile([P, P], BF16, tag="eqT")
                    if k % 3 == 0:
                        nc.vector.tensor_copy(eqT[:], scr[:])
                    else:
                        nc.scalar.copy(eqT[:], scr[:])
                    ops = next(pe_iter, None)
                    if ops:
                        for f in ops:
                            f()
                    nc.tensor.matmul(
                        out=dpsa[:, k * Hn:(k + 1) * Hn],
                        lhsT=eqT[:], rhs=d_sb[:, t * Hn:(t + 1) * Hn],
                        start=False, stop=True, skip_group_check=True)
            for rest in pe_iter:
                for f in rest:
                    f()

        def stage_w(t):
            """tsd -> leaky -> exp -> wfb -> S builds; returns the per-chunk
            aggregation matmul closures for interleaving."""
            st = state[t]
            ntot = sum(nch[t])
            agg = psA.tile([P, CO + (1 if merge_den else 0)], F32, tag="agg")
            st["agg"] = agg
            if merge_den:
                den_ap = agg[:, CO:CO + 1]
            else:
                den_t = ps1.tile([P, Hn], F32, tag="den")
                den_ap = den_t[:]
            st["den_ap"] = den_ap
            if ntot == 0:
                nc.vector.memset(agg[:], 0)
                if not merge_den:
                    nc.vector.memset(den_t[:], 0)
                return []
            eqa = st["eqa"]
            dpsa = st["dpsa"]
            Gs = st["G"]
            tsda = sbc.tile([P, max_ntot * Hn], F32, tag="tsda")
            b0 = 0
            for g in range(c.NG):
                n = nch[t][g]
                if n == 0:
                    continue
                Gv = Gs[g][:, 0:n * ELEM].rearrange(
                    "p (k d) -> p k d", d=ELEM)[:, :, CO:CO + Hn]
                Dv = dpsa[:, b0 * Hn:(b0 + n) * Hn].rearrange(
                    "p (k h) -> p k h", h=Hn)
                nc.vector.tensor_tensor(
                    out=tsda[:, b0 * Hn:(b0 + n) * Hn].rearrange(
                        "p (k h) -> p k h", h=Hn),
                    in0=Gv, in1=Dv, op=AL.add)
                b0 += n
            lra = sbc.tile([P, max_ntot * Hn], F32, tag="lra")
            nc.vector.scalar_tensor_tensor(
                out=lra[:, 0:ntot * Hn], in0=tsda[:, 0:ntot * Hn],
                scalar=NEG_SLOPE_ATT, in1=tsda[:, 0:ntot * Hn],
                op0=AL.mult, op1=AL.max)
            wfa = sbc.tile([P, max_ntot * Hn], F32, tag="wfa")
            nc.scalar.activation(wfa[:, 0:ntot * Hn], lra[:, 0:ntot * Hn],
                                 AF.Exp)
            wfb = sbc.tile([P, max_ntot * Hn], BF16, tag="wfb")
            nc.vector.tensor_copy(wfb[:, 0:ntot * Hn], wfa[:, 0:ntot * Hn])

            pe_ops = []
            gchunk = 0
            for g in range(c.NG):
                n = nch[t][g]
                if n == 0:
                    continue
                G = Gs[g]
                for b in range(n):
                    k = gchunk
                    first, last = (k == 0), (k == ntot - 1)
                    wf = wfa[:, k * Hn:(k + 1) * Hn]
                    wb = wfb[:, k * Hn:(k + 1) * Hn]
                    eq = eqa[:, k * P:(k + 1) * P]
                    S = sSp.tile([P, Hn * P], BF16, tag="S")
                    eq_b = bass.AP(eq.tensor, eq.offset,
                                   [list(eq.ap[0]), [0, Hn], list(eq.ap[1])])
                    nc.vector.tensor_tensor(
                        out=S[:].rearrange("p (h d) -> p h d", h=Hn),
                        in0=eq_b, in1=wf.to_broadcast([P, Hn, P]),
                        op=AL.mult)

                    def mk(S=S, G=G, b=b, eq=eq, wb=wb, first=first,
                           last=last):
                        def run():
                            BK = 512
                            for h in range(Hn):
                                h_first = (h * Ch) % BK == 0
                                h_last = ((h + 1) * Ch) % BK == 0 or (
                                    h == Hn - 1 and not merge_den)
                                nc.tensor.matmul(
                                    out=agg[:, h * Ch:(h + 1) * Ch],
                                    lhsT=S[:, h * P:(h + 1) * P],
                                    rhs=G[:, b * ELEM + h * Ch:
                                            b * ELEM + (h + 1) * Ch],
                                    start=first and h_first,
                                    stop=last and h_last)
                            if merge_den:
                                nc.tensor.matmul(out=den_ap, lhsT=eq,
                                                 rhs=wb[:, 0:1], start=False,
                                                 stop=last)
                            else:
                                nc.tensor.matmul(out=den_ap, lhsT=eq, rhs=wb,
                                                 start=first, stop=last)
                        return run

                    pe_ops.append([mk()])
                    gchunk += 1
            return pe_ops

        # pipeline: gather(t+2) | eqd(t+1) x agg(t) | w(t+1) | epilogue(t)
        stage_gather(0)
        stage_gather(1)
        stage_eqd(0, [])
        pend = stage_w(0)
        for t in range(c.T):
            if t + 2 < c.T:
                stage_gather(t + 2)
            if t + 1 < c.T:
                stage_eqd(t + 1, pend)
                pend = stage_w(t + 1)
            else:
                stage_eqd_flush(pend)
            _epilogue(nc, sb, ps1, c, meta, lay, t, state[t]["agg"],
                      state[t]["den_ap"], Hn, Ch, CO, bln_sb, id_sb,
                      fuse, final, y)
            del state[t]


def stage_eqd_flush(pend):
    for ops in pend:
        for f in ops:
            f()


def _epilogue(nc, sb, ps1, c, meta, lay, t, agg, den_ap, Hn, Ch, CO,
              bln_sb, id_sb, fuse, final, y):
    denr = sb.tile([P, Hn], F32, tag="denr")
    nc.vector.tensor_scalar(out=denr[:], in0=den_ap, scalar1=1e-16,
                            scalar2=None, op0=AL.add)
    rec = sb.tile([P, Hn], F32, tag="rec")
    nc.vector.reciprocal(rec[:], denr[:])
    ob = sb.tile([P, CO], F32 if final else BF16, tag="ob")
    for h in range(Hn):
        nc.vector.scalar_tensor_tensor(
            out=ob[:, h * Ch:(h + 1) * Ch], in0=agg[:, h * Ch:(h + 1) * Ch],
            scalar=rec[:, h:h + 1], op0=AL.mult,
            in1=bln_sb[:, h * Ch:(h + 1) * Ch], op1=AL.add)
    rs = sb.tile([P, 1], F32, tag="rs")
    nc.vector.tensor_reduce(out=rs[:], in_=ob[:], axis=mybir.AxisListType.X,
                            op=AL.add)
    nm = sb.tile([P, 1], F32, tag="nm")
    nc.vector.tensor_scalar(out=nm[:], in0=rs[:], scalar1=-1.0 / CO,
                            scalar2=None, op0=AL.mult)
    sqd = sb.tile([P, CO], BF16, tag="sqd")
    vs = sb.tile([P, 1], F32, tag="vs")
    nc.scalar.activation(sqd[:], ob[:], AF.Square, bias=nm[:, 0:1],
                         accum_out=vs[:])
    vstd = sb.tile([P, 1], F32, tag="vstd")
    nc.vector.tensor_scalar(out=vstd[:], in0=vs[:], scalar1=1.0 / CO,
                            scalar2=LN_EPS, op0=AL.mult, op1=AL.add)
    sd = sb.tile([P, 1], F32, tag="sd")
    nc.scalar.activation(sd[:], vstd[:], AF.Sqrt)
    rstd = sb.tile([P, 1], F32, tag="rstd")
    nc.vector.reciprocal(rstd[:], sd[:])
    xcs = sb.tile([P, CO], F32 if final else BF16, tag="xcs")
    nc.vector.tensor_scalar(out=xcs[:], in0=ob[:], scalar1=nm[:, 0:1],
                            scalar2=rstd[:, 0:1], op0=AL.add, op1=AL.mult)
    y1 = sb.tile([P, CO], F32 if final else BF16, tag="y1")
    nc.vector.tensor_tensor(out=y1[:], in0=xcs[:], in1=bln_sb[:, CO:2 * CO],
                            op=AL.mult)
    y2 = sb.tile([P, CO], F32 if final else BF16, tag="y2")
    nc.vector.tensor_tensor(out=y2[:], in0=y1[:], in1=bln_sb[:, 2 * CO:3 * CO],
                            op=AL.add)

    if final:
        mx = sb.tile([P, 1], F32, tag="mx")
        nc.vector.tensor_reduce(out=mx[:], in_=y2[:],
                                axis=mybir.AxisListType.X, op=AL.max)
        nmx = sb.tile([P, 1], F32, tag="nmx")
        nc.vector.tensor_scalar(out=nmx[:], in0=mx[:], scalar1=-1.0,
                                scalar2=None, op0=AL.mult)
        xs = sb.tile([P, CO], F32, tag="xs")
        nc.vector.tensor_scalar(out=xs[:], in0=y2[:], scalar1=nmx[:, 0:1],
                                scalar2=None, op0=AL.add)
        ex = sb.tile([P, CO], F32, tag="ex")
        se = sb.tile([P, 1], F32, tag="se")
        nc.scalar.activation(ex[:], xs[:], AF.Exp, accum_out=se[:])
        lse = sb.tile([P, 1], F32, tag="lse")
        nc.scalar.activation(lse[:], se[:], AF.Ln)
        nlse = sb.tile([P, 1], F32, tag="nlse")
        nc.vector.tensor_scalar(out=nlse[:], in0=lse[:], scalar1=-1.0,
                                scalar2=None, op0=AL.mult)
        yo = sb.tile([P, CO], F32, tag="yo")
        nc.vector.tensor_scalar(out=yo[:], in0=xs[:], scalar1=nlse[:, 0:1],
                                scalar2=None, op0=AL.add)
        nc.sync.dma_start(y[t * P:(t + 1) * P, :], yo[:])
        return

    x2 = sb.tile([P, CO], BF16, tag="x2")
    nc.vector.scalar_tensor_tensor(
        out=x2[:], in0=y2[:], scalar=NEG_SLOPE_ACT, in1=y2[:],
        op0=AL.mult, op1=AL.max)
    W_sb, KC, Ww = fuse["W_sb"], fuse["KC"], fuse["Ww"]
    CO2, Hn2, ELEMn = fuse["CO"], fuse["Hn2"], fuse["ELEMn"]
    xt2 = sb.tile([P, KC * P], BF16, tag="xt2")
    for k in range(KC):
        scr = ps1.tile([P, P], BF16, tag="scr")
        nc.tensor.transpose(out=scr[:], in_=x2[:, k * P:(k + 1) * P],
                            identity=id_sb[:])
        nc.scalar.copy(xt2[:, k * P:(k + 1) * P], scr[:])
    hp = ps1.tile([P, Ww], F32, tag="hnext")
    _mm_splits(nc, hp, xt2, W_sb, KC, Ww, P)
    _store_hs(nc, sb, hp[:, 0:CO2], hp[:, CO2:CO2 + Hn2],
              hp[:, CO2 + Hn2:CO2 + 2 * Hn2], CO2, Hn2, ELEMn,
              fuse["hs_loc"], fuse["d_next"], t)


# --------------------------------------------------------------------------
# entry point
# --------------------------------------------------------------------------

_CACHE = {}


def _get_nc(cfg, meta):
    key = (tuple(sorted((k, str(v)) for k, v in cfg.__dict__.items())),
           tuple(tuple(r) for r in meta.nch))
    if key not in _CACHE:
        _CACHE[key] = build_nc(cfg, meta)
    return _CACHE[key]


def kernel(**inputs):
    inputs = {k: np.asarray(v) for k, v in inputs.items()}
    x = inputs["x"]
    cfg = Cfg(N=x.shape[0], E=inputs["edge_src"].shape[0], F_IN=x.shape[1],
              HEADS=inputs["a_src1"].shape[0], C1=inputs["a_src1"].shape[1],
              C2=inputs["a_src2"].shape[1], NCLS=inputs["W3"].shape[0],
              NCORES=8)
    in_maps, meta = host_prep(cfg, **inputs)
    nc = _get_nc(cfg, meta)
    trace = bool(int(os.environ.get("GAT_TRACE", "0")))
    res = run_bass_kernel_spmd(nc, in_maps, core_ids=list(range(cfg.NCORES)),
                               trace=trace)
    global LAST_EXEC_NS, LAST_RES
    LAST_EXEC_NS = res.exec_time_ns
    LAST_RES = res
    out = np.concatenate(
        [res.results[cc]["y"][:cfg.NL] for cc in range(cfg.NCORES)], axis=0)
    return out.astype(np.float32)


LAST_EXEC_NS = None
LAST_RES = None


if __name__ == "__main__":
    pass

